# revision 1
# baseline (speedup 1.0000x reference)
"""ChebConv K=2 (L_hat = -D^-1/2 A D^-1/2) distributed over 8 NeuronCores.

Strategy (per spec sharding hint): nodes sharded 12500/core; edges partitioned
by destination shard. Two SPMD launches:

  L1 (per core, row-sharded edges): deg = segment_sum(w, row) via padded
     per-node weight table + free-dim reduce; dinv = deg>0 ? rsqrt(deg) : 0;
     Z = (dinv*x) @ W1 ; U = x @ W0 + b  for the core's node shard.
  host: concatenates Z shards -> Zfull (no arithmetic).
  L2 (per core, dest-sharded edges): for each 128-node output group,
     gather Z rows of edge sources (dma_gather, int16 indices bucketed by
     source range), build scaled one-hot S[e,slot] = -w_e * [slot==col_e]
     with one fused DVE op, accumulate S^T @ Zg in PSUM over edge tiles,
     out = dinv ⊙ psum + U.

Identity: out = x@W0 + b + dinv_col ⊙ (Σ_e 1[col=n](-w_e) (dinv⊙x)[row_e]) @ W1
        = x@W0 + Tx1@W1 + b with Tx1 = segment_sum(norm * x[row], col).
"""
import sys

if "/opt/trn_rl_repo" not in sys.path:
    sys.path.insert(0, "/opt/trn_rl_repo")

import numpy as np

import concourse.bass as bass
import concourse.bacc as bacc
import concourse.mybir as mybir
import concourse.tile as tile
from concourse.masks import make_identity
from concourse.bass_utils import run_bass_kernel_spmd

P = 128
D = 64
N_NODES = 100000
N_CORES = 8
NSH = N_NODES // N_CORES            # 12500 nodes per shard
NG = (NSH + P - 1) // P             # 98 groups per shard
BUCKET = 25000                      # z-table bucket rows (int16-addressable)
NBUCKETS = (N_NODES + BUCKET - 1) // BUCKET

_cache = {}
LAST_STATS = {}


# ----------------------------------------------------------------- L1 kernel
def build_l1(kd):
    nc = bacc.Bacc("TRN2", target_bir_lowering=False, debug=False,
                   num_devices=N_CORES)
    x_d = nc.dram_tensor("x", [NSH, D], mybir.dt.float32, kind="ExternalInput")
    wpad_d = nc.dram_tensor("wpad", [P, NG * kd], mybir.dt.float32,
                            kind="ExternalInput")
    w0_d = nc.dram_tensor("w0", [D, D], mybir.dt.float32, kind="ExternalInput")
    w1_d = nc.dram_tensor("w1", [D, D], mybir.dt.float32, kind="ExternalInput")
    bias_d = nc.dram_tensor("bias", [1, D], mybir.dt.float32, kind="ExternalInput")
    z_d = nc.dram_tensor("z", [NSH, D], mybir.dt.float32, kind="ExternalOutput")
    u_d = nc.dram_tensor("u", [NSH, D], mybir.dt.float32, kind="ExternalOutput")
    dinv_d = nc.dram_tensor("dinv", [P, NG], mybir.dt.float32, kind="ExternalOutput")

    with tile.TileContext(nc) as tc:
        with (
            tc.tile_pool(name="const", bufs=1) as cpool,
            tc.tile_pool(name="sbuf", bufs=4) as pool,
            tc.tile_pool(name="psum", bufs=2, space="PSUM") as psum_pool,
        ):
            ident = cpool.tile([P, P], mybir.dt.float32)
            make_identity(nc, ident[:])
            w0_t = cpool.tile([D, D], mybir.dt.float32)
            nc.sync.dma_start(w0_t[:], w0_d[:, :])
            w1_t = cpool.tile([D, D], mybir.dt.float32)
            nc.sync.dma_start(w1_t[:], w1_d[:, :])
            bias_t = cpool.tile([P, D], mybir.dt.float32)
            nc.sync.dma_start(bias_t[:], bias_d[:, :].to_broadcast([P, D]))

            # wpad arrives partition-major: wpad[p, g*kd + k] = weight slot k
            # of node g*128+p (zeros where node >= NSH or no edge).
            wbig = cpool.tile([P, NG * kd], mybir.dt.float32)
            nc.sync.dma_start(wbig[:], wpad_d[:, :])

            deg_t = cpool.tile([P, NG], mybir.dt.float32)
            for g in range(NG):
                nc.vector.reduce_sum(
                    deg_t[:, g:g + 1], wbig[:, g * kd:(g + 1) * kd],
                    axis=mybir.AxisListType.X,
                )
            m_t = cpool.tile([P, NG], mybir.dt.float32)
            nc.vector.tensor_scalar_max(m_t[:], deg_t[:], 1e-30)
            s_t = cpool.tile([P, NG], mybir.dt.float32)
            nc.scalar.activation(s_t[:], m_t[:], mybir.ActivationFunctionType.Sqrt)
            r_t = cpool.tile([P, NG], mybir.dt.float32)
            nc.vector.reciprocal(r_t[:], s_t[:])
            mask_t = cpool.tile([P, NG], mybir.dt.float32)
            nc.vector.tensor_scalar(
                out=mask_t[:], in0=deg_t[:], scalar1=0.0, scalar2=None,
                op0=mybir.AluOpType.is_gt,
            )
            dinv_t = cpool.tile([P, NG], mybir.dt.float32)
            nc.vector.tensor_mul(dinv_t[:], r_t[:], mask_t[:])
            nc.sync.dma_start(dinv_d[:, :], dinv_t[:])

            for g in range(NG):
                n0 = g * P
                n1 = min(n0 + P, NSH)
                np_ = n1 - n0
                x_t = pool.tile([P, D], mybir.dt.float32, tag="x")
                if np_ < P:
                    nc.vector.memset(x_t[:], 0.0)
                nc.sync.dma_start(x_t[:np_], x_d[n0:n1, :])
                y_t = pool.tile([P, D], mybir.dt.float32, tag="y")
                nc.scalar.activation(
                    y_t[:], x_t[:], mybir.ActivationFunctionType.Copy,
                    scale=dinv_t[:, g:g + 1],
                )
                xT_p = psum_pool.tile([D, P], mybir.dt.float32, tag="xTp", space="PSUM")
                nc.tensor.transpose(out=xT_p[:], in_=x_t[:], identity=ident[:])
                xT_t = pool.tile([D, P], mybir.dt.float32, tag="xT")
                nc.vector.tensor_copy(xT_t[:], xT_p[:])
                yT_p = psum_pool.tile([D, P], mybir.dt.float32, tag="yTp", space="PSUM")
                nc.tensor.transpose(out=yT_p[:], in_=y_t[:], identity=ident[:])
                yT_t = pool.tile([D, P], mybir.dt.float32, tag="yT")
                nc.vector.tensor_copy(yT_t[:], yT_p[:])
                z_p = psum_pool.tile([P, D], mybir.dt.float32, tag="zp", space="PSUM")
                nc.tensor.matmul(out=z_p[:], lhsT=yT_t[:], rhs=w1_t[:],
                                 start=True, stop=True)
                z_t = pool.tile([P, D], mybir.dt.float32, tag="z")
                nc.vector.tensor_copy(z_t[:], z_p[:])
                nc.sync.dma_start(z_d[n0:n1, :], z_t[:np_])
                u_p = psum_pool.tile([P, D], mybir.dt.float32, tag="up", space="PSUM")
                nc.tensor.matmul(out=u_p[:], lhsT=xT_t[:], rhs=w0_t[:],
                                 start=True, stop=True)
                u_t = pool.tile([P, D], mybir.dt.float32, tag="u")
                nc.vector.tensor_add(u_t[:], u_p[:], bias_t[:])
                nc.sync.dma_start(u_d[n0:n1, :], u_t[:np_])
    nc.compile()
    return nc


# ----------------------------------------------------------------- L2 kernel
def build_l2(sched):
    """sched: tuple of NG tuples of NBUCKETS ints (multiples of 128)."""
    tot_tiles = sum(s // P for gs in sched for s in gs)
    tot16 = sum(s // 16 for gs in sched for s in gs)

    nc = bacc.Bacc("TRN2", target_bir_lowering=False, debug=False,
                   num_devices=N_CORES)
    z_d = nc.dram_tensor("zfull", [N_NODES, D], mybir.dt.float32,
                         kind="ExternalInput")
    u_d = nc.dram_tensor("u", [NSH, D], mybir.dt.float32, kind="ExternalInput")
    dinv_d = nc.dram_tensor("dinv", [P, NG], mybir.dt.float32,
                            kind="ExternalInput")
    gidx_d = nc.dram_tensor("gidx", [P, tot16], mybir.dt.int16,
                            kind="ExternalInput")
    slot_d = nc.dram_tensor("slot", [P, tot_tiles], mybir.dt.float32,
                            kind="ExternalInput")
    negw_d = nc.dram_tensor("negw", [P, tot_tiles], mybir.dt.float32,
                            kind="ExternalInput")
    iota_d = nc.dram_tensor("iota", [P, P], mybir.dt.float32,
                            kind="ExternalInput")
    out_d = nc.dram_tensor("out", [NSH, D], mybir.dt.float32,
                           kind="ExternalOutput")

    with tile.TileContext(nc) as tc:
        with (
            tc.tile_pool(name="const", bufs=1) as cpool,
            tc.tile_pool(name="sbuf", bufs=3) as pool,
            tc.tile_pool(name="gpool", bufs=2) as gpool,
            tc.tile_pool(name="psum", bufs=4, space="PSUM") as psum_pool,
        ):
            iota_t = cpool.tile([P, P], mybir.dt.float32)
            nc.sync.dma_start(iota_t[:], iota_d[:, :])
            dinv_t = cpool.tile([P, NG], mybir.dt.float32)
            nc.sync.dma_start(dinv_t[:], dinv_d[:, :])
            # slot/negw metadata resident in SBUF
            slot_t = cpool.tile([P, tot_tiles], mybir.dt.float32)
            nc.sync.dma_start(slot_t[:], slot_d[:, :])
            negw_t = cpool.tile([P, tot_tiles], mybir.dt.float32)
            nc.sync.dma_start(negw_t[:], negw_d[:, :])

            off16 = 0
            tile_off = 0
            for g in range(NG):
                n0 = g * P
                n1 = min(n0 + P, NSH)
                np_ = n1 - n0
                tg = sum(s // P for s in sched[g])
                if tg > 0:
                    gbuf = gpool.tile([P, tg, D], mybir.dt.float32, tag="gbuf")
                    idx_t = gpool.tile([P, sum(s // 16 for s in sched[g])],
                                       mybir.dt.int16, tag="idx")
                    nc.sync.dma_start(
                        idx_t[:], gidx_d[:, off16:off16 + idx_t.shape[1]])
                    # gathers: one call per non-empty source bucket
                    i16 = 0
                    t0 = 0
                    for b in range(NBUCKETS):
                        s = sched[g][b]
                        if s == 0:
                            continue
                        b0 = b * BUCKET
                        b1 = min(b0 + BUCKET, N_NODES)
                        nc.gpsimd.dma_gather(
                            out_ap=gbuf[:, t0:t0 + s // P, :],
                            in_ap=z_d[b0:b1, :],
                            idxs_ap=idx_t[:, i16:i16 + s // 16],
                            num_idxs=s,
                            num_idxs_reg=s,
                            elem_size=D,
                        )
                        i16 += s // 16
                        t0 += s // P
                    psum = psum_pool.tile([P, D], mybir.dt.float32, tag="acc",
                                          space="PSUM")
                    for t in range(tg):
                        s_t = pool.tile([P, P], mybir.dt.float32, tag="onehot")
                        nc.vector.tensor_scalar(
                            out=s_t[:],
                            in0=iota_t[:],
                            scalar1=slot_t[:, tile_off + t:tile_off + t + 1],
                            scalar2=negw_t[:, tile_off + t:tile_off + t + 1],
                            op0=mybir.AluOpType.is_equal,
                            op1=mybir.AluOpType.mult,
                        )
                        nc.tensor.matmul(
                            out=psum[:],
                            lhsT=s_t[:],
                            rhs=gbuf[:, t, :],
                            start=(t == 0),
                            stop=(t == tg - 1),
                        )
                u_t = pool.tile([P, D], mybir.dt.float32, tag="u")
                nc.sync.dma_start(u_t[:np_], u_d[n0:n1, :])
                o_t = pool.tile([P, D], mybir.dt.float32, tag="o")
                if tg > 0:
                    nc.scalar.activation(
                        o_t[:], psum[:], mybir.ActivationFunctionType.Copy,
                        scale=dinv_t[:, g:g + 1],
                    )
                    nc.vector.tensor_add(o_t[:], o_t[:], u_t[:])
                else:
                    nc.vector.tensor_copy(o_t[:], u_t[:])
                nc.sync.dma_start(out_d[n0:n1, :], o_t[:np_])
                off16 += sum(s // 16 for s in sched[g])
                tile_off += tg
    nc.compile()
    return nc


# ------------------------------------------------------------- host prep
def _prep_l1(row, w):
    """Per-core padded weight tables. Returns list of [P, NG*kd] arrays."""
    core = row // NSH
    kd_per_core = []
    data = []
    for c in range(N_CORES):
        sel = core == c
        r_loc = (row[sel] - c * NSH).astype(np.int64)
        w_c = w[sel]
        counts = np.bincount(r_loc, minlength=NSH)
        kd_per_core.append(int(counts.max()) if counts.size else 0)
        data.append((r_loc, w_c, counts))
    kd = max(kd_per_core)
    kd = max(4, ((kd + 3) // 4) * 4)
    out = []
    for r_loc, w_c, counts in data:
        offs = np.cumsum(counts) - counts
        order = np.argsort(r_loc, kind="stable")
        r_s = r_loc[order]
        w_s = w_c[order]
        k = np.arange(len(r_s)) - offs[r_s]
        wpad = np.zeros((NSH, kd), np.float32)
        wpad[r_s, k] = w_s
        # partition-major layout [P, NG*kd]: row p, col g*kd+k = node g*128+p
        wbig = np.zeros((NG * P, kd), np.float32)
        wbig[:NSH] = wpad
        wbig = wbig.reshape(NG, P, kd).transpose(1, 0, 2).reshape(P, NG * kd)
        out.append(np.ascontiguousarray(wbig))
    return kd, out


def _prep_l2(row, col, w):
    """Per-core L2 metadata. Returns (sched, per-core dict arrays)."""
    core = col // NSH
    percore = []
    counts = np.zeros((N_CORES, NG, NBUCKETS), np.int64)
    for c in range(N_CORES):
        sel = core == c
        rows = row[sel]
        col_loc = col[sel] - c * NSH
        w_c = w[sel]
        g = col_loc // P
        slot = col_loc % P
        b = rows // BUCKET
        rel = rows % BUCKET
        order = np.lexsort((rel, b, g))
        g, slot, b, rel, w_c = g[order], slot[order], b[order], rel[order], w_c[order]
        cnt = np.bincount(g * NBUCKETS + b, minlength=NG * NBUCKETS)
        counts[c] = cnt.reshape(NG, NBUCKETS)
        percore.append((g, slot, b, rel, w_c))
    smax = counts.max(axis=0)
    sched = tuple(
        tuple(int(-(-smax[g, b] // P) * P) if smax[g, b] > 0 else 0
              for b in range(NBUCKETS))
        for g in range(NG)
    )
    tot = sum(s for gs in sched for s in gs)
    tot_tiles = tot // P
    tot16 = tot // 16

    arrays = []
    for c in range(N_CORES):
        g, slot, b, rel, w_c = percore[c]
        gidx = np.zeros(tot, np.int16)
        slots = np.zeros(tot, np.float32)
        negw = np.zeros(tot, np.float32)
        # per (g,b) segment offsets in the padded stream
        seg_sizes = np.array([[sched[gi][bi] for bi in range(NBUCKETS)]
                              for gi in range(NG)], np.int64).reshape(-1)
        seg_offs = np.cumsum(seg_sizes) - seg_sizes
        seg_id = g * NBUCKETS + b
        cnt = counts[c].reshape(-1)
        # position within segment for each edge (edges sorted by seg_id)
        offs_e = np.cumsum(cnt) - cnt
        pos_in_seg = np.arange(len(g)) - offs_e[seg_id]
        pos = seg_offs[seg_id] + pos_in_seg
        gidx[pos] = rel.astype(np.int16)
        slots[pos] = slot.astype(np.float32)
        negw[pos] = -w_c
        # wrap indices: per call (segment), seq j -> [j%16, j//16], tiled to 128
        gw = np.zeros((P, tot16), np.int16)
        sw = np.zeros((P, tot_tiles), np.float32)
        nw = np.zeros((P, tot_tiles), np.float32)
        for sid in range(NG * NBUCKETS):
            s = seg_sizes[sid]
            if s == 0:
                continue
            o = seg_offs[sid]
            seg = gidx[o:o + s]
            wr = seg.reshape(s // 16, 16).T  # [16, s/16]
            gw[:, o // 16:(o + s) // 16] = np.tile(wr, (8, 1))
        sw[:, :] = slots.reshape(tot_tiles, P).T
        nw[:, :] = negw.reshape(tot_tiles, P).T
        arrays.append({"gidx": gw, "slot": sw, "negw": nw})
    return sched, arrays


# ------------------------------------------------------------------ kernel()
def kernel(x, edge_index, edge_weight, W0, W1, b):
    global LAST_STATS
    x = np.asarray(x, np.float32)
    edge_index = np.asarray(edge_index)
    w = np.asarray(edge_weight, np.float32)
    W0 = np.asarray(W0, np.float32)
    W1 = np.asarray(W1, np.float32)
    b = np.asarray(b, np.float32)
    row = edge_index[0].astype(np.int64)
    col = edge_index[1].astype(np.int64)

    kd, wpads = _prep_l1(row, w)
    sched, l2arr = _prep_l2(row, col, w)

    if ("l1", kd) not in _cache:
        _cache[("l1", kd)] = build_l1(kd)
    nc1 = _cache[("l1", kd)]
    if ("l2", sched) not in _cache:
        _cache[("l2", sched)] = build_l2(sched)
    nc2 = _cache[("l2", sched)]

    bias2d = b.reshape(1, D)
    in1 = [
        {"x": np.ascontiguousarray(x[c * NSH:(c + 1) * NSH]),
         "wpad": wpads[c], "w0": W0, "w1": W1, "bias": bias2d}
        for c in range(N_CORES)
    ]
    res1 = run_bass_kernel_spmd(nc1, in1, core_ids=list(range(N_CORES)))
    zfull = np.concatenate([res1.results[c]["z"] for c in range(N_CORES)], axis=0)
    iota = np.tile(np.arange(P, dtype=np.float32), (P, 1))
    in2 = [
        {"zfull": zfull, "u": res1.results[c]["u"],
         "dinv": res1.results[c]["dinv"],
         "gidx": l2arr[c]["gidx"], "slot": l2arr[c]["slot"],
         "negw": l2arr[c]["negw"], "iota": iota}
        for c in range(N_CORES)
    ]
    res2 = run_bass_kernel_spmd(nc2, in2, core_ids=list(range(N_CORES)))
    out = np.concatenate([res2.results[c]["out"] for c in range(N_CORES)], axis=0)
    LAST_STATS = {
        "l1_exec_ns": res1.exec_time_ns,
        "l2_exec_ns": res2.exec_time_ns,
        "sched_tiles": sum(s // P for gs in sched for s in gs),
    }
    return out.astype(np.float32)


# revision 4
# speedup vs baseline: 1.5527x; 1.5527x over previous
"""ChebConv K=2 (L_hat = -D^-1/2 A D^-1/2) distributed over 8 NeuronCores.

Sharding (per spec hint): nodes 12500/core; edges partitioned by destination
shard. Two SPMD launches:

  L1 (row-sharded edges): deg = segment_sum(w, row) via a padded per-node
     weight table + free-dim reduce; dinv = deg>0 ? rsqrt(deg) : 0;
     Z = dinv ⊙ (x @ W1) in fp16; U = x @ W0 + b. All per node shard.
  host: concatenates Z shards -> Zfull (layout only, no arithmetic).
  L2 (dest-sharded edges): per 128-node output group, gather Z rows of edge
     sources (dma_gather fp16, int16 indices bucketed by source range, two
     SWDGE queues), build the scaled one-hot S[e,slot] = -w_e * [slot==col_e]
     with one fused DVE op, accumulate S^T @ Zg in PSUM (fp16 matmuls) over
     edge tiles, out = dinv ⊙ psum + U.

Identity: out = x@W0 + b + dinv_col ⊙ Σ_e 1[col=n](-w_e)(dinv⊙(x@W1))[row_e]
        = x@W0 + Tx1@W1 + b with Tx1 = segment_sum(norm * x[row], col).

Edge schedule is equalized across cores (segment sizes = max over cores) so
one SPMD kernel serves all 8 cores; per-core shortfall is padded with index 0
and weight 0. Gather calls merge 8 groups ("super-groups") per source bucket;
tiles straddling group boundaries are processed once per group with the other
group's edges masked (weight 0).
"""
import sys

if "/opt/trn_rl_repo" not in sys.path:
    sys.path.insert(0, "/opt/trn_rl_repo")

import numpy as np

import concourse.bass as bass
import concourse.bacc as bacc
import concourse.mybir as mybir
import concourse.tile as tile
from concourse.bass_utils import run_bass_kernel_spmd

P = 128
D = 64
N_NODES = 100000
N_CORES = 8
NSH = N_NODES // N_CORES            # 12500 nodes per shard
NG = (NSH + P - 1) // P             # 98 groups per shard
SG_GROUPS = 8                       # groups per gather super-call
NSG = (NG + SG_GROUPS - 1) // SG_GROUPS
BUCKET = 25000                      # z-table bucket rows (int16-addressable)
NBUCKETS = (N_NODES + BUCKET - 1) // BUCKET

F32 = mybir.dt.float32
F16 = mybir.dt.float16
I16 = mybir.dt.int16

_cache = {}
LAST_STATS = {}


# ----------------------------------------------------------------- L1 kernel
def build_l1(kd):
    nc = bacc.Bacc("TRN2", target_bir_lowering=False, debug=False,
                   num_devices=N_CORES)
    xt_d = nc.dram_tensor("xt", [D, NSH], F16, kind="ExternalInput")
    wpad_d = nc.dram_tensor("wpad", [P, NG * kd], F32, kind="ExternalInput")
    w0_d = nc.dram_tensor("w0", [D, D], F16, kind="ExternalInput")
    w1_d = nc.dram_tensor("w1", [D, D], F16, kind="ExternalInput")
    bias_d = nc.dram_tensor("bias", [1, D], F32, kind="ExternalInput")
    z_d = nc.dram_tensor("z", [NSH, D], F16, kind="ExternalOutput")
    u_d = nc.dram_tensor("u", [NSH, D], F32, kind="ExternalOutput")
    dinv_d = nc.dram_tensor("dinv", [P, NG], F32, kind="ExternalOutput")

    with tile.TileContext(nc) as tc:
        with (
            tc.tile_pool(name="const", bufs=1) as cpool,
            tc.tile_pool(name="sbuf", bufs=4) as pool,
            tc.tile_pool(name="psum", bufs=2, space="PSUM") as psum_pool,
        ):
            w0_t = cpool.tile([D, D], F16)
            nc.sync.dma_start(w0_t[:], w0_d[:, :])
            w1_t = cpool.tile([D, D], F16)
            nc.sync.dma_start(w1_t[:], w1_d[:, :])
            bias_t = cpool.tile([P, D], F32)
            nc.sync.dma_start(bias_t[:], bias_d[:, :].to_broadcast([P, D]))
            # xT resident: [64, 12500] fp16 = 25KB/partition on 64 partitions
            xt_t = cpool.tile([D, NSH], F16)
            nc.sync.dma_start(xt_t[:], xt_d[:, :])
            wbig = cpool.tile([P, NG * kd], F32)
            nc.sync.dma_start(wbig[:], wpad_d[:, :])

            deg_t = cpool.tile([P, NG], F32)
            for g in range(NG):
                nc.vector.reduce_sum(
                    deg_t[:, g:g + 1], wbig[:, g * kd:(g + 1) * kd],
                    axis=mybir.AxisListType.X,
                )
            m_t = cpool.tile([P, NG], F32)
            nc.vector.tensor_scalar_max(m_t[:], deg_t[:], 1e-30)
            s_t = cpool.tile([P, NG], F32)
            nc.scalar.activation(s_t[:], m_t[:], mybir.ActivationFunctionType.Sqrt)
            r_t = cpool.tile([P, NG], F32)
            nc.vector.reciprocal(r_t[:], s_t[:])
            mask_t = cpool.tile([P, NG], F32)
            nc.vector.tensor_scalar(
                out=mask_t[:], in0=deg_t[:], scalar1=0.0, scalar2=None,
                op0=mybir.AluOpType.is_gt,
            )
            dinv_t = cpool.tile([P, NG], F32)
            nc.vector.tensor_mul(dinv_t[:], r_t[:], mask_t[:])
            nc.sync.dma_start(dinv_d[:, :], dinv_t[:])

            for g in range(NG):
                n0 = g * P
                n1 = min(n0 + P, NSH)
                np_ = n1 - n0
                v_p = psum_pool.tile([P, D], F32, tag="vp", space="PSUM")
                nc.tensor.matmul(out=v_p[:np_], lhsT=xt_t[:, n0:n1],
                                 rhs=w1_t[:], start=True, stop=True)
                z_t = pool.tile([P, D], F16, tag="z")
                nc.scalar.activation(
                    z_t[:np_], v_p[:np_], mybir.ActivationFunctionType.Copy,
                    scale=dinv_t[:np_, g:g + 1],
                )
                nc.sync.dma_start(z_d[n0:n1, :], z_t[:np_])
                u_p = psum_pool.tile([P, D], F32, tag="up", space="PSUM")
                nc.tensor.matmul(out=u_p[:np_], lhsT=xt_t[:, n0:n1],
                                 rhs=w0_t[:], start=True, stop=True)
                u_t = pool.tile([P, D], F32, tag="u")
                nc.vector.tensor_add(u_t[:np_], u_p[:np_], bias_t[:np_])
                nc.sync.dma_start(u_d[n0:n1, :], u_t[:np_])
    nc.compile()
    return nc


# ----------------------------------------------------------------- L2 kernel
def build_l2(sched):
    """sched: static schedule, same for all cores.

    sched = (calls, instances, tot16, tot_tiles)
      calls: tuple per (sg, b) of (num_idxs, valid, i16_off, tile_off, bucket)
             num_idxs/valid in edges; i16_off into gidx cols; tile_off into
             the sg's gather buffer.
      instances: tuple per group of tuples (global_tile, meta_col) where
             global_tile indexes (sg, tile-in-sg) flattened.
      sg_tiles: tuple of tiles per sg.
    """
    calls, instances, sg_tiles, tot16, tot_meta = sched
    max_sg_tiles = max(sg_tiles)

    nc = bacc.Bacc("TRN2", target_bir_lowering=False, debug=False,
                   num_devices=N_CORES, num_swdge_queues=2)
    z_d = nc.dram_tensor("zfull", [N_NODES, 2 * D], F16, kind="ExternalInput")
    u_d = nc.dram_tensor("u", [NSH, D], F32, kind="ExternalInput")
    dinv_d = nc.dram_tensor("dinv", [P, NG], F32, kind="ExternalInput")
    gidx_d = nc.dram_tensor("gidx", [P, tot16], I16, kind="ExternalInput")
    slot_d = nc.dram_tensor("slot", [P, tot_meta], F32, kind="ExternalInput")
    negw_d = nc.dram_tensor("negw", [P, tot_meta], F32, kind="ExternalInput")
    iota_d = nc.dram_tensor("iota", [P, P], F16, kind="ExternalInput")
    out_d = nc.dram_tensor("out", [NSH, D], F32, kind="ExternalOutput")

    with tile.TileContext(nc) as tc:
        with (
            tc.tile_pool(name="const", bufs=1) as cpool,
            tc.tile_pool(name="sbuf", bufs=4) as pool,
            tc.tile_pool(name="meta", bufs=2) as mpool,
            tc.tile_pool(name="psum", bufs=4, space="PSUM") as psum_pool,
        ):
            iota_t = cpool.tile([P, P], F16)
            nc.sync.dma_start(iota_t[:], iota_d[:, :])
            dinv_t = cpool.tile([P, NG], F32)
            nc.sync.dma_start(dinv_t[:], dinv_d[:, :])
            slot_t = cpool.tile([P, tot_meta], F32)
            nc.sync.dma_start(slot_t[:], slot_d[:, :])
            negw_t = cpool.tile([P, tot_meta], F32)
            nc.sync.dma_start(negw_t[:], negw_d[:, :])
            gbufs = [cpool.tile([P, max_sg_tiles, 2 * D], F16, name=f"gbuf{i}")
                     for i in range(2)]
            nc.vector.memset(gbufs[0][:], 0.0)
            nc.vector.memset(gbufs[1][:], 0.0)

            for sg in range(NSG):
                g0 = sg * SG_GROUPS
                g1 = min(g0 + SG_GROUPS, NG)
                gbuf = gbufs[sg % 2]
                sg_calls = [c for c in calls if c[0] == sg]
                i16_lo = min(c[3] for c in sg_calls)
                i16_hi = max(c[3] + c[1] // 16 for c in sg_calls)
                idx_t = mpool.tile([P, i16_hi - i16_lo], I16, tag="idx")
                nc.sync.dma_start(idx_t[:], gidx_d[:, i16_lo:i16_hi])
                for (csg, num_idxs, valid, i16_off, tile_off, b) in sg_calls:
                    b0 = b * BUCKET
                    b1 = min(b0 + BUCKET, N_NODES)
                    nc.gpsimd.dma_gather(
                        out_ap=gbuf[:, tile_off:tile_off + num_idxs // P, :],
                        in_ap=z_d[b0:b1, :],
                        idxs_ap=idx_t[:, i16_off - i16_lo:
                                      i16_off - i16_lo + num_idxs // 16],
                        num_idxs=num_idxs,
                        num_idxs_reg=valid,
                        elem_size=2 * D,
                        single_packet=False,
                        queue_num=b % 2,
                    )
                for g in range(g0, g1):
                    insts = instances[g]
                    n0 = g * P
                    n1 = min(n0 + P, NSH)
                    np_ = n1 - n0
                    u_t = pool.tile([P, D], F32, tag="u")
                    nc.sync.dma_start(u_t[:np_], u_d[n0:n1, :])
                    o_t = pool.tile([P, D], F32, tag="o")
                    if insts:
                        psum = psum_pool.tile([P, D], F32, tag="acc",
                                              space="PSUM")
                        for k, (ltile, mcol) in enumerate(insts):
                            s_t = pool.tile([P, P], F16, tag="onehot")
                            nc.vector.tensor_scalar(
                                out=s_t[:],
                                in0=iota_t[:],
                                scalar1=slot_t[:, mcol:mcol + 1],
                                scalar2=negw_t[:, mcol:mcol + 1],
                                op0=mybir.AluOpType.is_equal,
                                op1=mybir.AluOpType.mult,
                            )
                            nc.tensor.matmul(
                                out=psum[:],
                                lhsT=s_t[:],
                                rhs=gbuf[:, ltile, :D],
                                start=(k == 0),
                                stop=(k == len(insts) - 1),
                            )
                        nc.scalar.activation(
                            o_t[:np_], psum[:np_],
                            mybir.ActivationFunctionType.Copy,
                            scale=dinv_t[:np_, g:g + 1],
                        )
                        nc.vector.tensor_add(o_t[:np_], o_t[:np_], u_t[:np_])
                    else:
                        nc.vector.tensor_copy(o_t[:np_], u_t[:np_])
                    nc.sync.dma_start(out_d[n0:n1, :], o_t[:np_])
    nc.compile()
    return nc


# ------------------------------------------------------------- host prep
def _prep_l1(row, w):
    """Per-core padded weight tables. Returns (kd, list of [P, NG*kd])."""
    core = row // NSH
    data = []
    kd = 4
    for c in range(N_CORES):
        sel = core == c
        r_loc = (row[sel] - c * NSH).astype(np.int64)
        w_c = w[sel]
        counts = np.bincount(r_loc, minlength=NSH)
        kd = max(kd, int(counts.max()))
        data.append((r_loc, w_c, counts))
    kd = ((kd + 3) // 4) * 4
    out = []
    for r_loc, w_c, counts in data:
        offs = np.cumsum(counts) - counts
        order = np.argsort(r_loc, kind="stable")
        r_s = r_loc[order]
        w_s = w_c[order]
        k = np.arange(len(r_s)) - offs[r_s]
        wpad = np.zeros((NG * P, kd), np.float32)
        wpad[r_s, k] = w_s
        wbig = wpad.reshape(NG, P, kd).transpose(1, 0, 2).reshape(P, NG * kd)
        out.append(np.ascontiguousarray(wbig))
    return kd, out


def _prep_l2(row, col, w):
    """Builds the core-equalized L2 schedule + per-core data arrays."""
    core = col // NSH
    percore = []
    counts = np.zeros((N_CORES, NG, NBUCKETS), np.int64)
    for c in range(N_CORES):
        sel = core == c
        rows = row[sel]
        col_loc = col[sel] - c * NSH
        w_c = w[sel]
        g = col_loc // P
        slot = col_loc % P
        b = rows // BUCKET
        rel = rows % BUCKET
        order = np.lexsort((rel, b, g))
        percore.append((g[order], slot[order], b[order], rel[order], w_c[order]))
        cnt = np.bincount(g * NBUCKETS + b, minlength=NG * NBUCKETS)
        counts[c] = cnt.reshape(NG, NBUCKETS)
    smax = counts.max(axis=0)          # [NG, NBUCKETS] equalized segment sizes

    # --- static schedule ---
    calls = []        # (sg, num_idxs, valid, i16_off, tile_off, bucket)
    seg_pos = np.zeros((NG, NBUCKETS), np.int64)   # start of segment in call
    seg_call = np.zeros((NG, NBUCKETS), np.int64)  # call id of segment
    sg_tiles = []
    i16_off = 0
    for sg in range(NSG):
        g0, g1 = sg * SG_GROUPS, min((sg + 1) * SG_GROUPS, NG)
        toff = 0
        for b in range(NBUCKETS):
            valid = int(smax[g0:g1, b].sum())
            if valid == 0:
                continue
            num_idxs = -(-valid // P) * P
            pos = 0
            for g in range(g0, g1):
                seg_pos[g, b] = pos
                seg_call[g, b] = len(calls)
                pos += int(smax[g, b])
            calls.append((sg, num_idxs, valid, i16_off, toff, b))
            i16_off += num_idxs // 16
            toff += num_idxs // P
        sg_tiles.append(toff)
    tot16 = i16_off
    max_sg_tiles = max(sg_tiles)

    # instances per group: (local_tile_in_sg, meta_col)
    instances = []
    meta_col = 0
    inst_meta = []    # (g, b, local_tile, seg_a, seg_len)
    for g in range(NG):
        insts = []
        sg = g // SG_GROUPS
        for b in range(NBUCKETS):
            s = int(smax[g, b])
            if s == 0:
                continue
            cid = seg_call[g, b]
            _, num_idxs, valid, _, tile_off, _ = calls[cid]
            a = int(seg_pos[g, b])
            t0 = a // P
            t1 = -(-(a + s) // P)
            for t in range(t0, t1):
                insts.append((tile_off + t, meta_col))
                inst_meta.append((g, b, tile_off + t, a, s, cid))
                meta_col += 1
        instances.append(tuple(insts))
    tot_meta = meta_col

    sched = (tuple(calls), tuple(instances), tuple(sg_tiles), tot16, tot_meta)

    # --- per-core arrays ---
    arrays = []
    call_list = calls
    for c in range(N_CORES):
        g_e, slot_e, b_e, rel_e, w_e = percore[c]
        cnt = counts[c]
        # edge positions inside the equalized segments
        seg_id = g_e * NBUCKETS + b_e
        cnt_flat = cnt.reshape(-1)
        offs_e = np.cumsum(cnt_flat) - cnt_flat
        pos_in_seg = np.arange(len(g_e)) - offs_e[seg_id]
        # absolute position within the call's valid region
        abs_pos = seg_pos.reshape(-1)[seg_id] + pos_in_seg
        call_of_e = seg_call.reshape(-1)[seg_id]

        # per-call index sequences
        gidx = np.zeros((P, tot16), np.int16)
        # meta arrays
        slots = np.zeros((P, tot_meta), np.float32)
        negw = np.zeros((P, tot_meta), np.float32)

        for cid, (sg, num_idxs, valid, i16o, tile_off, b) in enumerate(call_list):
            sel = call_of_e == cid
            seq = np.zeros(num_idxs, np.int64)
            seq[valid:] = -1
            seq[abs_pos[sel]] = rel_e[sel]
            wr = seq.reshape(num_idxs // 16, 16).T.astype(np.int16)
            gidx[:, i16o:i16o + num_idxs // 16] = np.tile(wr, (8, 1))

        arrays.append({"gidx": gidx, "_slots": slots, "_negw": negw,
                       "_gsb": (g_e, slot_e, b_e, rel_e, w_e, abs_pos, call_of_e)})

    # vectorized meta fill: map each edge to its instance meta column
    # build lookup: (cid, local_tile, g) -> meta_col
    inst_lookup = {}
    for mcol, (g, b, ltile, a, s, cid) in enumerate(inst_meta):
        sg, num_idxs, valid, i16o, tile_off, _ = call_list[cid]
        inst_lookup[(cid, ltile - tile_off, g)] = mcol
    for c in range(N_CORES):
        g_e, slot_e, b_e, rel_e, w_e, abs_pos, call_of_e = arrays[c]["_gsb"]
        slots = arrays[c]["_slots"]
        negw = arrays[c]["_negw"]
        tloc = abs_pos // P
        p_of_e = abs_pos % P
        keys = np.stack([call_of_e, tloc, g_e], axis=1)
        # map via dict (1.6M/8 lookups, vectorize with np.unique)
        uk, inv = np.unique(keys, axis=0, return_inverse=True)
        mcols = np.array([inst_lookup[(int(a), int(b_), int(g_))]
                          for a, b_, g_ in uk], np.int64)
        mc_e = mcols[inv]
        slots[p_of_e, mc_e] = slot_e.astype(np.float32)
        negw[p_of_e, mc_e] = -w_e
        arrays[c] = {"gidx": arrays[c]["gidx"], "slot": slots, "negw": negw}
    return sched, arrays


# ------------------------------------------------------------------ kernel()
def kernel(x, edge_index, edge_weight, W0, W1, b):
    global LAST_STATS
    x = np.asarray(x, np.float32)
    edge_index = np.asarray(edge_index)
    w = np.asarray(edge_weight, np.float32)
    W0 = np.asarray(W0, np.float32)
    W1 = np.asarray(W1, np.float32)
    b = np.asarray(b, np.float32)
    row = edge_index[0].astype(np.int64)
    col = edge_index[1].astype(np.int64)

    kd, wpads = _prep_l1(row, w)
    sched, l2arr = _prep_l2(row, col, w)
    sched_key = (sched[0], sched[2], sched[3], sched[4])

    if ("l1", kd) not in _cache:
        _cache[("l1", kd)] = build_l1(kd)
    nc1 = _cache[("l1", kd)]
    if ("l2", sched_key) not in _cache:
        _cache[("l2", sched_key)] = build_l2(sched)
    nc2 = _cache[("l2", sched_key)]

    bias2d = b.reshape(1, D)
    w0h = W0.astype(np.float16)
    w1h = W1.astype(np.float16)
    in1 = [
        {"xt": np.ascontiguousarray(
            x[c * NSH:(c + 1) * NSH].T.astype(np.float16)),
         "wpad": wpads[c], "w0": w0h, "w1": w1h, "bias": bias2d}
        for c in range(N_CORES)
    ]
    res1 = run_bass_kernel_spmd(nc1, in1, core_ids=list(range(N_CORES)))
    zfull = np.concatenate([res1.results[c]["z"] for c in range(N_CORES)], axis=0)
    zfull2 = np.ascontiguousarray(np.concatenate([zfull, zfull], axis=1))
    iota = np.tile(np.arange(P, dtype=np.float16), (P, 1))
    in2 = [
        {"zfull": zfull2, "u": res1.results[c]["u"],
         "dinv": res1.results[c]["dinv"],
         "gidx": l2arr[c]["gidx"], "slot": l2arr[c]["slot"],
         "negw": l2arr[c]["negw"], "iota": iota}
        for c in range(N_CORES)
    ]
    res2 = run_bass_kernel_spmd(nc2, in2, core_ids=list(range(N_CORES)))
    out = np.concatenate([res2.results[c]["out"] for c in range(N_CORES)], axis=0)
    LAST_STATS = {
        "l1_exec_ns": res1.exec_time_ns,
        "l2_exec_ns": res2.exec_time_ns,
        "descs": sum(c[2] for c in sched[0]),
        "tiles": sched[4],
    }
    return out.astype(np.float32)


# revision 5
# speedup vs baseline: 1.8738x; 1.2068x over previous
"""ChebConv K=2 (L_hat = -D^-1/2 A D^-1/2) distributed over 8 NeuronCores.

Sharding (per spec hint): nodes 12500/core; edges partitioned by destination
shard. Two SPMD launches:

  L1 (row-sharded edges): deg = segment_sum(w, row) via a padded per-node
     weight table + free-dim reduce; dinv = deg>0 ? rsqrt(deg) : 0;
     Z = dinv ⊙ (x @ W1) in fp16; U = x @ W0 + b. All per node shard.
  host: concatenates Z shards -> Zfull (layout only, no arithmetic).
  L2 (dest-sharded edges): per 128-node output group, gather Z rows of edge
     sources (dma_gather fp16, int16 indices bucketed by source range, two
     SWDGE queues), build the scaled one-hot S[e,slot] = -w_e * [slot==col_e]
     with one fused DVE op, accumulate S^T @ Zg in PSUM (fp16 matmuls) over
     edge tiles, out = dinv ⊙ psum + U.

Identity: out = x@W0 + b + dinv_col ⊙ Σ_e 1[col=n](-w_e)(dinv⊙(x@W1))[row_e]
        = x@W0 + Tx1@W1 + b with Tx1 = segment_sum(norm * x[row], col).

Edge schedule is equalized across cores (segment sizes = max over cores) so
one SPMD kernel serves all 8 cores; per-core shortfall is padded with index 0
and weight 0. Gather calls merge 8 groups ("super-groups") per source bucket;
tiles straddling group boundaries are processed once per group with the other
group's edges masked (weight 0).
"""
import sys

if "/opt/trn_rl_repo" not in sys.path:
    sys.path.insert(0, "/opt/trn_rl_repo")

import numpy as np

import concourse.bass as bass
import concourse.bacc as bacc
import concourse.mybir as mybir
import concourse.tile as tile
from concourse.bass_utils import run_bass_kernel_spmd

P = 128
D = 64
N_NODES = 100000
N_CORES = 8
NSH = N_NODES // N_CORES            # 12500 nodes per shard
NG = (NSH + P - 1) // P             # 98 groups per shard
SG_GROUPS = 8                       # groups per gather super-call
NSG = (NG + SG_GROUPS - 1) // SG_GROUPS
BUCKET = 25000                      # z-table bucket rows (int16-addressable)
NBUCKETS = (N_NODES + BUCKET - 1) // BUCKET

F32 = mybir.dt.float32
F16 = mybir.dt.float16
I16 = mybir.dt.int16

_cache = {}
LAST_STATS = {}


# ----------------------------------------------------------------- L1 kernel
def build_l1(kd):
    nc = bacc.Bacc("TRN2", target_bir_lowering=False, debug=False,
                   num_devices=N_CORES)
    xt_d = nc.dram_tensor("xt", [D, NSH], F16, kind="ExternalInput")
    wpad_d = nc.dram_tensor("wpad", [P, NG * kd], F32, kind="ExternalInput")
    w0_d = nc.dram_tensor("w0", [D, D], F16, kind="ExternalInput")
    w1_d = nc.dram_tensor("w1", [D, D], F16, kind="ExternalInput")
    bias_d = nc.dram_tensor("bias", [1, D], F32, kind="ExternalInput")
    z_d = nc.dram_tensor("z", [NSH, D], F16, kind="ExternalOutput")
    u_d = nc.dram_tensor("u", [NSH, D], F32, kind="ExternalOutput")
    dinv_d = nc.dram_tensor("dinv", [P, NG], F32, kind="ExternalOutput")

    with tile.TileContext(nc) as tc:
        with (
            tc.tile_pool(name="const", bufs=1) as cpool,
            tc.tile_pool(name="sbuf", bufs=4) as pool,
            tc.tile_pool(name="psum", bufs=2, space="PSUM") as psum_pool,
        ):
            w0_t = cpool.tile([D, D], F16)
            nc.sync.dma_start(w0_t[:], w0_d[:, :])
            w1_t = cpool.tile([D, D], F16)
            nc.sync.dma_start(w1_t[:], w1_d[:, :])
            bias_t = cpool.tile([P, D], F32)
            nc.sync.dma_start(bias_t[:], bias_d[:, :].to_broadcast([P, D]))
            # xT resident: [64, 12500] fp16 = 25KB/partition on 64 partitions
            xt_t = cpool.tile([D, NSH], F16)
            nc.sync.dma_start(xt_t[:], xt_d[:, :])
            wbig = cpool.tile([P, NG * kd], F32)
            nc.sync.dma_start(wbig[:], wpad_d[:, :])

            deg_t = cpool.tile([P, NG], F32)
            for g in range(NG):
                nc.vector.reduce_sum(
                    deg_t[:, g:g + 1], wbig[:, g * kd:(g + 1) * kd],
                    axis=mybir.AxisListType.X,
                )
            m_t = cpool.tile([P, NG], F32)
            nc.vector.tensor_scalar_max(m_t[:], deg_t[:], 1e-30)
            s_t = cpool.tile([P, NG], F32)
            nc.scalar.activation(s_t[:], m_t[:], mybir.ActivationFunctionType.Sqrt)
            r_t = cpool.tile([P, NG], F32)
            nc.vector.reciprocal(r_t[:], s_t[:])
            mask_t = cpool.tile([P, NG], F32)
            nc.vector.tensor_scalar(
                out=mask_t[:], in0=deg_t[:], scalar1=0.0, scalar2=None,
                op0=mybir.AluOpType.is_gt,
            )
            dinv_t = cpool.tile([P, NG], F32)
            nc.vector.tensor_mul(dinv_t[:], r_t[:], mask_t[:])
            nc.sync.dma_start(dinv_d[:, :], dinv_t[:])

            for g in range(NG):
                n0 = g * P
                n1 = min(n0 + P, NSH)
                np_ = n1 - n0
                v_p = psum_pool.tile([P, D], F32, tag="vp", space="PSUM")
                nc.tensor.matmul(out=v_p[:np_], lhsT=xt_t[:, n0:n1],
                                 rhs=w1_t[:], start=True, stop=True)
                z_t = pool.tile([P, D], F16, tag="z")
                nc.scalar.activation(
                    z_t[:np_], v_p[:np_], mybir.ActivationFunctionType.Copy,
                    scale=dinv_t[:np_, g:g + 1],
                )
                nc.sync.dma_start(z_d[n0:n1, :], z_t[:np_])
                u_p = psum_pool.tile([P, D], F32, tag="up", space="PSUM")
                nc.tensor.matmul(out=u_p[:np_], lhsT=xt_t[:, n0:n1],
                                 rhs=w0_t[:], start=True, stop=True)
                u_t = pool.tile([P, D], F32, tag="u")
                nc.vector.tensor_add(u_t[:np_], u_p[:np_], bias_t[:np_])
                nc.sync.dma_start(u_d[n0:n1, :], u_t[:np_])
    nc.compile()
    return nc


# ----------------------------------------------------------------- L2 kernel
def build_l2(sched):
    """sched: static schedule, same for all cores.

    sched = (calls, instances, tot16, tot_tiles)
      calls: tuple per (sg, b) of (num_idxs, valid, i16_off, tile_off, bucket)
             num_idxs/valid in edges; i16_off into gidx cols; tile_off into
             the sg's gather buffer.
      instances: tuple per group of tuples (global_tile, meta_col) where
             global_tile indexes (sg, tile-in-sg) flattened.
      sg_tiles: tuple of tiles per sg.
    """
    calls, instances, sg_tiles, tot16, tot_meta = sched
    max_sg_tiles = max(sg_tiles)

    nc = bacc.Bacc("TRN2", target_bir_lowering=False, debug=False,
                   num_devices=N_CORES, num_swdge_queues=2)
    z_d = nc.dram_tensor("zfull", [N_NODES, 2 * D], F16, kind="ExternalInput")
    u_d = nc.dram_tensor("u", [NSH, D], F32, kind="ExternalInput")
    dinv_d = nc.dram_tensor("dinv", [P, NG], F32, kind="ExternalInput")
    gidx_d = nc.dram_tensor("gidx", [P, tot16], I16, kind="ExternalInput")
    slot_d = nc.dram_tensor("slot", [P, tot_meta], F16, kind="ExternalInput")
    negw_d = nc.dram_tensor("negw", [P, tot_meta], F16, kind="ExternalInput")
    iota_d = nc.dram_tensor("iota", [P, P], F16, kind="ExternalInput")
    out_d = nc.dram_tensor("out", [NSH, D], F32, kind="ExternalOutput")

    with tile.TileContext(nc) as tc:
        with (
            tc.tile_pool(name="const", bufs=1) as cpool,
            tc.tile_pool(name="sbuf", bufs=4) as pool,
            tc.tile_pool(name="meta", bufs=2) as mpool,
            tc.tile_pool(name="psum", bufs=4, space="PSUM") as psum_pool,
        ):
            iota_t = cpool.tile([P, P], F16)
            nc.sync.dma_start(iota_t[:], iota_d[:, :])
            dinv_t = cpool.tile([P, NG], F32)
            nc.sync.dma_start(dinv_t[:], dinv_d[:, :])
            slot_t = cpool.tile([P, tot_meta], F16)
            nc.sync.dma_start(slot_t[:], slot_d[:, :])
            negw_t = cpool.tile([P, tot_meta], F16)
            nc.sync.dma_start(negw_t[:], negw_d[:, :])
            gbufs = [cpool.tile([P, max_sg_tiles, 2 * D], F16, name=f"gbuf{i}")
                     for i in range(2)]
            nc.vector.memset(gbufs[0][:], 0.0)
            nc.vector.memset(gbufs[1][:], 0.0)

            for sg in range(NSG):
                g0 = sg * SG_GROUPS
                g1 = min(g0 + SG_GROUPS, NG)
                gbuf = gbufs[sg % 2]
                sg_calls = [c for c in calls if c[0] == sg]
                i16_lo = min(c[3] for c in sg_calls)
                i16_hi = max(c[3] + c[1] // 16 for c in sg_calls)
                idx_t = mpool.tile([P, i16_hi - i16_lo], I16, tag="idx")
                nc.sync.dma_start(idx_t[:], gidx_d[:, i16_lo:i16_hi])
                for (csg, num_idxs, valid, i16_off, tile_off, b) in sg_calls:
                    b0 = b * BUCKET
                    b1 = min(b0 + BUCKET, N_NODES)
                    nc.gpsimd.dma_gather(
                        out_ap=gbuf[:, tile_off:tile_off + num_idxs // P, :],
                        in_ap=z_d[b0:b1, :],
                        idxs_ap=idx_t[:, i16_off - i16_lo:
                                      i16_off - i16_lo + num_idxs // 16],
                        num_idxs=num_idxs,
                        num_idxs_reg=valid,
                        elem_size=2 * D,
                        single_packet=False,
                        queue_num=b % 2,
                    )
                for g in range(g0, g1):
                    runs = instances[g]
                    n0 = g * P
                    n1 = min(n0 + P, NSH)
                    np_ = n1 - n0
                    u_t = pool.tile([P, D], F32, tag="u")
                    nc.sync.dma_start(u_t[:np_], u_d[n0:n1, :])
                    o_t = pool.tile([P, D], F32, tag="o")
                    if runs:
                        psum = psum_pool.tile([P, D], F32, tag="acc",
                                              space="PSUM")
                        ninst = sum(r[2] for r in runs)
                        k = 0
                        for (t0, m0, kb) in runs:
                            sw = pool.tile([P, kb, P], F16, tag="swide")
                            ia = iota_t[:]
                            in0 = bass.AP(ia.tensor, ia.offset,
                                          [ia.ap[0], [0, kb], ia.ap[1]])
                            sa = slot_t[:, m0:m0 + kb]
                            in1 = bass.AP(sa.tensor, sa.offset,
                                          [sa.ap[0], sa.ap[1], [0, P]])
                            nc.vector.tensor_tensor(
                                out=sw[:], in0=in0, in1=in1,
                                op=mybir.AluOpType.is_equal)
                            gs = pool.tile([P, kb, D], F16, tag="gsc")
                            na = negw_t[:, m0:m0 + kb]
                            in1b = bass.AP(na.tensor, na.offset,
                                           [na.ap[0], na.ap[1], [0, D]])
                            nc.vector.tensor_tensor(
                                out=gs[:], in0=gbuf[:, t0:t0 + kb, 0:D],
                                in1=in1b, op=mybir.AluOpType.mult)
                            for j in range(kb):
                                nc.tensor.matmul(
                                    out=psum[:],
                                    lhsT=sw[:, j, :],
                                    rhs=gs[:, j, :],
                                    start=(k == 0),
                                    stop=(k == ninst - 1),
                                )
                                k += 1
                        nc.scalar.activation(
                            o_t[:np_], psum[:np_],
                            mybir.ActivationFunctionType.Copy,
                            scale=dinv_t[:np_, g:g + 1],
                        )
                        nc.vector.tensor_add(o_t[:np_], o_t[:np_], u_t[:np_])
                    else:
                        nc.vector.tensor_copy(o_t[:np_], u_t[:np_])
                    nc.sync.dma_start(out_d[n0:n1, :], o_t[:np_])
    nc.compile()
    return nc


# ------------------------------------------------------------- host prep
def _prep_l1(row, w):
    """Per-core padded weight tables. Returns (kd, list of [P, NG*kd])."""
    core = row // NSH
    data = []
    kd = 4
    for c in range(N_CORES):
        sel = core == c
        r_loc = (row[sel] - c * NSH).astype(np.int64)
        w_c = w[sel]
        counts = np.bincount(r_loc, minlength=NSH)
        kd = max(kd, int(counts.max()))
        data.append((r_loc, w_c, counts))
    kd = ((kd + 3) // 4) * 4
    out = []
    for r_loc, w_c, counts in data:
        offs = np.cumsum(counts) - counts
        order = np.argsort(r_loc, kind="stable")
        r_s = r_loc[order]
        w_s = w_c[order]
        k = np.arange(len(r_s)) - offs[r_s]
        wpad = np.zeros((NG * P, kd), np.float32)
        wpad[r_s, k] = w_s
        wbig = wpad.reshape(NG, P, kd).transpose(1, 0, 2).reshape(P, NG * kd)
        out.append(np.ascontiguousarray(wbig))
    return kd, out


def _prep_l2(row, col, w):
    """Builds the core-equalized L2 schedule + per-core data arrays."""
    core = col // NSH
    percore = []
    counts = np.zeros((N_CORES, NG, NBUCKETS), np.int64)
    for c in range(N_CORES):
        sel = core == c
        rows = row[sel]
        col_loc = col[sel] - c * NSH
        w_c = w[sel]
        g = col_loc // P
        slot = col_loc % P
        b = rows // BUCKET
        rel = rows % BUCKET
        order = np.lexsort((rel, b, g))
        percore.append((g[order], slot[order], b[order], rel[order], w_c[order]))
        cnt = np.bincount(g * NBUCKETS + b, minlength=NG * NBUCKETS)
        counts[c] = cnt.reshape(NG, NBUCKETS)
    smax = counts.max(axis=0)          # [NG, NBUCKETS] equalized segment sizes

    # --- static schedule ---
    calls = []        # (sg, num_idxs, valid, i16_off, tile_off, bucket)
    seg_pos = np.zeros((NG, NBUCKETS), np.int64)   # start of segment in call
    seg_call = np.zeros((NG, NBUCKETS), np.int64)  # call id of segment
    sg_tiles = []
    i16_off = 0
    for sg in range(NSG):
        g0, g1 = sg * SG_GROUPS, min((sg + 1) * SG_GROUPS, NG)
        toff = 0
        for b in range(NBUCKETS):
            valid = int(smax[g0:g1, b].sum())
            if valid == 0:
                continue
            num_idxs = -(-valid // P) * P
            pos = 0
            for g in range(g0, g1):
                seg_pos[g, b] = pos
                seg_call[g, b] = len(calls)
                pos += int(smax[g, b])
            calls.append((sg, num_idxs, valid, i16_off, toff, b))
            i16_off += num_idxs // 16
            toff += num_idxs // P
        sg_tiles.append(toff)
    tot16 = i16_off
    max_sg_tiles = max(sg_tiles)

    # instances per group: runs of (gbuf_tile0, meta_col0, ntiles)
    instances = []
    meta_col = 0
    inst_meta = []    # (g, b, local_tile, seg_a, seg_len, call_id)
    for g in range(NG):
        runs = []
        for b in range(NBUCKETS):
            s = int(smax[g, b])
            if s == 0:
                continue
            cid = seg_call[g, b]
            _, num_idxs, valid, _, tile_off, _ = calls[cid]
            a = int(seg_pos[g, b])
            t0 = a // P
            t1 = -(-(a + s) // P)
            runs.append((tile_off + t0, meta_col, t1 - t0))
            for t in range(t0, t1):
                inst_meta.append((g, b, tile_off + t, a, s, cid))
                meta_col += 1
        instances.append(tuple(runs))
    tot_meta = meta_col

    sched = (tuple(calls), tuple(instances), tuple(sg_tiles), tot16, tot_meta)

    # --- per-core arrays ---
    arrays = []
    call_list = calls
    for c in range(N_CORES):
        g_e, slot_e, b_e, rel_e, w_e = percore[c]
        cnt = counts[c]
        # edge positions inside the equalized segments
        seg_id = g_e * NBUCKETS + b_e
        cnt_flat = cnt.reshape(-1)
        offs_e = np.cumsum(cnt_flat) - cnt_flat
        pos_in_seg = np.arange(len(g_e)) - offs_e[seg_id]
        # absolute position within the call's valid region
        abs_pos = seg_pos.reshape(-1)[seg_id] + pos_in_seg
        call_of_e = seg_call.reshape(-1)[seg_id]

        # per-call index sequences
        gidx = np.zeros((P, tot16), np.int16)
        # meta arrays
        slots = np.zeros((P, tot_meta), np.float16)
        negw = np.zeros((P, tot_meta), np.float16)

        for cid, (sg, num_idxs, valid, i16o, tile_off, b) in enumerate(call_list):
            sel = call_of_e == cid
            seq = np.zeros(num_idxs, np.int64)
            seq[valid:] = -1
            seq[abs_pos[sel]] = rel_e[sel]
            wr = seq.reshape(num_idxs // 16, 16).T.astype(np.int16)
            gidx[:, i16o:i16o + num_idxs // 16] = np.tile(wr, (8, 1))

        arrays.append({"gidx": gidx, "_slots": slots, "_negw": negw,
                       "_gsb": (g_e, slot_e, b_e, rel_e, w_e, abs_pos, call_of_e)})

    # vectorized meta fill: map each edge to its instance meta column
    # build lookup: (cid, local_tile, g) -> meta_col
    inst_lookup = {}
    for mcol, (g, b, ltile, a, s, cid) in enumerate(inst_meta):
        sg, num_idxs, valid, i16o, tile_off, _ = call_list[cid]
        inst_lookup[(cid, ltile - tile_off, g)] = mcol
    for c in range(N_CORES):
        g_e, slot_e, b_e, rel_e, w_e, abs_pos, call_of_e = arrays[c]["_gsb"]
        slots = arrays[c]["_slots"]
        negw = arrays[c]["_negw"]
        tloc = abs_pos // P
        p_of_e = abs_pos % P
        keys = np.stack([call_of_e, tloc, g_e], axis=1)
        # map via dict (1.6M/8 lookups, vectorize with np.unique)
        uk, inv = np.unique(keys, axis=0, return_inverse=True)
        mcols = np.array([inst_lookup[(int(a), int(b_), int(g_))]
                          for a, b_, g_ in uk], np.int64)
        mc_e = mcols[inv]
        slots[p_of_e, mc_e] = slot_e.astype(np.float16)
        negw[p_of_e, mc_e] = (-w_e).astype(np.float16)
        arrays[c] = {"gidx": arrays[c]["gidx"], "slot": slots, "negw": negw}
    return sched, arrays


# ------------------------------------------------------------------ kernel()
def kernel(x, edge_index, edge_weight, W0, W1, b):
    global LAST_STATS
    x = np.asarray(x, np.float32)
    edge_index = np.asarray(edge_index)
    w = np.asarray(edge_weight, np.float32)
    W0 = np.asarray(W0, np.float32)
    W1 = np.asarray(W1, np.float32)
    b = np.asarray(b, np.float32)
    row = edge_index[0].astype(np.int64)
    col = edge_index[1].astype(np.int64)

    kd, wpads = _prep_l1(row, w)
    sched, l2arr = _prep_l2(row, col, w)
    sched_key = (sched[0], sched[2], sched[3], sched[4])

    if ("l1", kd) not in _cache:
        _cache[("l1", kd)] = build_l1(kd)
    nc1 = _cache[("l1", kd)]
    if ("l2", sched_key) not in _cache:
        _cache[("l2", sched_key)] = build_l2(sched)
    nc2 = _cache[("l2", sched_key)]

    bias2d = b.reshape(1, D)
    w0h = W0.astype(np.float16)
    w1h = W1.astype(np.float16)
    in1 = [
        {"xt": np.ascontiguousarray(
            x[c * NSH:(c + 1) * NSH].T.astype(np.float16)),
         "wpad": wpads[c], "w0": w0h, "w1": w1h, "bias": bias2d}
        for c in range(N_CORES)
    ]
    res1 = run_bass_kernel_spmd(nc1, in1, core_ids=list(range(N_CORES)))
    zfull = np.concatenate([res1.results[c]["z"] for c in range(N_CORES)], axis=0)
    zfull2 = np.ascontiguousarray(np.concatenate([zfull, zfull], axis=1))
    iota = np.tile(np.arange(P, dtype=np.float16), (P, 1))
    in2 = [
        {"zfull": zfull2, "u": res1.results[c]["u"],
         "dinv": res1.results[c]["dinv"],
         "gidx": l2arr[c]["gidx"], "slot": l2arr[c]["slot"],
         "negw": l2arr[c]["negw"], "iota": iota}
        for c in range(N_CORES)
    ]
    res2 = run_bass_kernel_spmd(nc2, in2, core_ids=list(range(N_CORES)))
    out = np.concatenate([res2.results[c]["out"] for c in range(N_CORES)], axis=0)
    LAST_STATS = {
        "l1_exec_ns": res1.exec_time_ns,
        "l2_exec_ns": res2.exec_time_ns,
        "descs": sum(c[2] for c in sched[0]),
        "tiles": sched[4],
    }
    return out.astype(np.float32)
